# revision 51
# baseline (speedup 1.0000x reference)
"""Trainium2 Bass kernel for a 16-expert top-4 MoE layer with shared expert.

Strategy (8 NeuronCores, expert-parallel):
  - Each core owns 2 experts. The host pairs a high-count expert with a
    low-count one (balanced pairing) and compiles with slot capacities
    C0/C1 rounded up from the actual max counts (runtime-adaptive
    compile constants), instead of a fixed worst-case capacity.
  - The router is replicated on every core and must reproduce the fp32
    top-4 selection exactly: the gate matrix is sent as a split-fp16
    pair [g16 | dg16] (stationary, 128 x 32) against an fp16 activation
    stream x16.  The residual-x term is dropped (host-verified: the
    selection margin is >> the dropped term for this scheme), so the
    router stream is half the bytes of a full split.
  - Activations stream in 8 chunks of 256 tokens, HOST-PERMUTED so that
    chunk 0 is always the core's own shared-expert token slice; the
    chunk doubles as the shared-expert input (no separate xTs tensor).
    A tokmap input gives the absolute token id per (partition, block).
  - Dispatch tables are built on-chip with the PE: per (block, expert) a
    one-hot matrix from a DVE is_eq against a column-iota, accumulated
    as [tok|g]^T @ onehot in PSUM.  Slot positions come from a
    strict-upper-triangular prefix matmul plus a cross-block running
    count.  Per-block DVE work is batched across blocks where possible.
  - Each expert gathers its C_e token rows (fp16) with a single
    indirect DMA ([128, NS]-offset), transposes them on the PE to
    [H, C_e], computes SwiGLU in fp16, scales rows by the table's
    routing weight on the Vector engine, and writes compact [C_e, H]
    fp16 rows.  The host applies out[tok] += y per expert using the
    [2, C_e] table, so there is no on-device output scatter.
  - The shared expert is token-sliced: core c computes tokens
    [256c, 256(c+1)); its matmuls fill the dispatch window.
"""

import numpy as np

import concourse.bass as bass
import concourse.mybir as mybir
import concourse.tile as tile
from concourse import bacc
from concourse.bass import IndirectOffsetOnAxis
from concourse.bass_utils import run_bass_kernel_spmd
from concourse.masks import make_identity, make_upper_triangular

FP32 = mybir.dt.float32
FP16 = mybir.dt.float16
I32 = mybir.dt.int32

T = 2048
H = 1024
II = 1024  # intermediate size
E = 16
TOPK = 4
NCORES = 8
EPC = 2            # experts per core
TSH = T // NCORES  # shared-expert tokens per core
NCH = 8            # router activation chunks
CW = T // NCH      # tokens per chunk (= TSH)
NBLK = T // 128    # token blocks
KO = H // 128      # contraction subtiles

# The hardware ACT engine has a Silu LUT; CoreSim does not implement it.
USE_SILU = True

_compiled = {}


def _seg512(c):
    """Column segments of width <= 512 covering [0, c)."""
    return [(i, min(512, c - i)) for i in range(0, c, 512)]


def _slots(c):
    """(slot index, row offset, rows) tiles of 128 covering [0, c)."""
    return [(s, s * 128, min(128, c - s * 128)) for s in range((c + 127) // 128)]


def _build(use_silu, caps):
    C0, C1 = caps
    CAP = [C0, C1]
    CM = max(C0, C1)
    nc = bacc.Bacc(None, target_bir_lowering=False, debug=False)

    # ---- I/O ----
    xT16 = nc.dram_tensor("xT16", [NCH, 128, KO, CW], FP16, kind="ExternalInput")
    x16 = nc.dram_tensor("x16", [T, H], FP16, kind="ExternalInput")
    tokmap = nc.dram_tensor("tokmap", [128, NBLK], I32, kind="ExternalInput")
    gwt = nc.dram_tensor("gwt", [128, KO, 2 * E], FP16, kind="ExternalInput")
    identd_in = nc.dram_tensor("identd_in", [128, E], FP32, kind="ExternalInput")
    bias_bc = nc.dram_tensor("bias_bc", [128, 4, E], FP32, kind="ExternalInput")
    w1t = nc.dram_tensor("w1t", [EPC, 128, KO, II], FP16, kind="ExternalInput")
    w3t = nc.dram_tensor("w3t", [EPC, 128, KO, II], FP16, kind="ExternalInput")
    w2t = nc.dram_tensor("w2t", [EPC, 128, KO, H], FP16, kind="ExternalInput")
    sw1t = nc.dram_tensor("sw1t", [128, KO, II], FP16, kind="ExternalInput")
    sw3t = nc.dram_tensor("sw3t", [128, KO, II], FP16, kind="ExternalInput")
    sw2t = nc.dram_tensor("sw2t", [128, KO, H], FP16, kind="ExternalInput")

    ye = [
        nc.dram_tensor(f"ye{e}", [CAP[e], H], FP16, kind="ExternalOutput")
        for e in range(EPC)
    ]
    tbl = [
        nc.dram_tensor(f"tbl{e}", [2, CAP[e]], FP32, kind="ExternalOutput")
        for e in range(EPC)
    ]
    ysh = nc.dram_tensor("ysh", [TSH, H], FP16, kind="ExternalOutput")
    trash = nc.dram_tensor("trash", [1, 512], FP32, kind="ExternalOutput")

    def silu_into(dst, src):
        """dst(f16) = silu(src); src is a PSUM fp32 tile."""
        if use_silu:
            nc.scalar.activation(dst, src, mybir.ActivationFunctionType.Silu)
        else:
            nc.scalar.activation(dst, src, mybir.ActivationFunctionType.Sigmoid)
            nc.vector.tensor_tensor(dst, dst, src, mybir.AluOpType.mult)

    with tile.TileContext(nc) as tc:
        with (
            tc.tile_pool(name="const", bufs=1) as const,
            tc.tile_pool(name="apool", bufs=5) as apool,
            tc.tile_pool(name="small", bufs=3) as small,
            tc.tile_pool(name="state", bufs=1) as state,
            tc.tile_pool(name="wpool", bufs=2) as wpool,
            tc.tile_pool(name="w2pool", bufs=2) as w2pool,
            tc.tile_pool(name="bpool", bufs=2) as bpool,
            tc.tile_pool(name="bigpool", bufs=2) as bigpool,
            tc.tile_pool(name="upool", bufs=2) as upool,
            tc.tile_pool(name="xgpool", bufs=1) as xgpool,
            tc.tile_pool(name="ypool", bufs=1) as ypool,
            tc.tile_pool(name="ohpool", bufs=3) as ohpool,
            tc.tile_pool(name="psum", bufs=2, space="PSUM") as psum,
            tc.tile_pool(name="ptbl", bufs=1, space="PSUM") as ptbl,
            tc.tile_pool(name="psum4", bufs=4, space="PSUM") as psum4,
        ):
            # ---------- critical-path DMAs first ----------
            # chunk 0 doubles as the shared-expert activation slice; it is
            # pinned for the whole kernel.  The sync queue carries ONLY the
            # router activation chunks so they stream back-to-back.
            xts = const.tile([128, KO, CW], FP16)
            nc.sync.dma_start(xts[:], xT16[0])
            gwt_sb = const.tile([128, KO, 2 * E], FP16)
            nc.gpsimd.dma_start(gwt_sb[:], gwt[:, :, :])
            bias_sb = const.tile([128, 4, E], FP32)
            nc.gpsimd.dma_start(bias_sb[:], bias_bc[:, :, :])
            toki = const.tile([128, NBLK], I32)
            nc.gpsimd.dma_start(toki[:], tokmap[:, :])

            # ---------- constants ----------
            lones = const.tile([128, 128], FP16)
            nc.gpsimd.memset(lones[:], 1.0)
            warm = const.tile([128, 512], FP16)
            nc.vector.memset(warm[:], 1.0)
            ltri = const.tile([128, 128], FP16)
            make_upper_triangular(nc, ltri[:], val=1.0, diag=False)  # k<m strictly
            ident32 = const.tile([128, 128], FP32)
            make_identity(nc, ident32[:])
            identf = const.tile([128, 128], FP16)
            make_identity(nc, identf[:])
            # stacked identity (I every 16 rows): the block transpose matmul
            # sums the g16*x and dg16*x logit halves for free, and works from
            # any 32-row band of the packed router PSUM tile
            identd = const.tile([128, E], FP32)
            nc.gpsimd.dma_start(identd[:], identd_in[:, :])

            tokf = const.tile([128, NBLK], FP16)
            nc.vector.tensor_copy(tokf[:], toki[:])
            coli = state.tile([128, CM], I32)
            nc.gpsimd.iota(coli[:], pattern=[[1, CM]], base=0, channel_multiplier=0)
            colf = const.tile([128, CM], FP16)
            nc.vector.tensor_copy(colf[:], coli[:])

            # per-expert [token | routing weight] columns
            tokg = []
            for e in range(EPC):
                tg = state.tile([128, NBLK, 2], FP16, tag=f"tokg{e}")
                nc.vector.tensor_copy(tg[:, :, 0], tokf[:])
                tokg.append(tg)

            ush = bpool.tile([128, KO, TSH], FP16, tag="ush")

            # PE warmup: ramps the clock gate while the chunk-0 DMA lands
            wu_ps = psum4.tile([128, 512], FP32, tag="mm")
            for w in range(10):
                nc.tensor.matmul(
                    wu_ps[:], lhsT=lones[:], rhs=warm[:],
                    start=(w == 0), stop=(w == 9),
                )
            wu_sb = small.tile([128, 16], FP32, tag="warm")
            nc.vector.tensor_copy(wu_sb[:], wu_ps[:, :16])
            nc.gpsimd.dma_start(trash[0:1, :16], wu_sb[:1, :])

            logitsT = state.tile([2 * E, T], FP32)
            logit_all = state.tile([128, NBLK, E], FP32)
            m16_all = state.tile([128, E, NBLK], FP16)
            msum_all = state.tile([128, E, NBLK], FP16)
            expt_all = state.tile([128, NBLK, E], FP32)
            ssum_all = state.tile([128, NBLK], FP32)
            rcp_all = state.tile([128, NBLK], FP32)
            slotf_all = state.tile([128, EPC, NBLK], FP32)

            # ---------- phase A1: router matmuls ----------
            # 4 chunks accumulate into one PSUM tile at 32-row bands; one
            # full-width DVE copy drains each tile
            ps_lt = None
            for ch in range(NCH):
                if ch == 0:
                    xd_c = xts
                else:
                    xd_c = apool.tile([128, KO, CW], FP16, tag="xd")
                    nc.sync.dma_start(xd_c[:], xT16[ch])
                ps_lt = psum.tile([2 * E, CW], FP32, tag="pslt")
                for ko in range(KO):
                    nc.tensor.matmul(
                        ps_lt[:],
                        lhsT=gwt_sb[:, ko, :],
                        rhs=xd_c[:, ko, :],
                        start=(ko == 0),
                        stop=(ko == KO - 1),
                    )
                nc.vector.tensor_copy(
                    logitsT[:, ch * CW : (ch + 1) * CW], ps_lt[:]
                )

            # weight DMAs: min-time paced so their traffic lands behind the
            # router stream. IMPORTANT: emitted in pace order — a paced DMA
            # at the head of the queue blocks everything emitted after it.
            with tc.tile_wait_until(0.010):
                sw1s = wpool.tile([128, KO, II], FP16, tag="w1")
                nc.scalar.dma_start(sw1s[:], sw1t[:, :, :])
            with tc.tile_wait_until(0.013):
                sw3s = wpool.tile([128, KO, II], FP16, tag="w3")
                nc.scalar.dma_start(sw3s[:], sw3t[:, :, :])
            with tc.tile_wait_until(0.016):
                w1s = wpool.tile([128, KO, II], FP16, tag="w1")
                nc.scalar.dma_start(w1s[:], w1t[0])
            with tc.tile_wait_until(0.019):
                w3s = wpool.tile([128, KO, II], FP16, tag="w3")
                nc.scalar.dma_start(w3s[:], w3t[0])
            with tc.tile_wait_until(0.024):
                sw2s = w2pool.tile([128, KO, H], FP16, tag="w2")
                nc.scalar.dma_start(sw2s[:], sw2t[:, :, :])

            # ---------- dispatch: blocks -> positions -> tables, pipelined
            # per group of GB blocks so tables finish right behind the router
            GB = 4
            erow = [0, 32]  # expert row offset in the shared table PSUM bank
            ps_ta = ptbl.tile([erow[-1] + 2, 512], FP32, tag="ta")
            ps_tb = {}
            for e in range(EPC):
                if CAP[e] > 512:
                    ps_tbe = ptbl.tile([2, CAP[e] - 512], FP32, tag=f"tb{e}")
                    ps_tb[e] = ps_tbe

            for j0 in range(0, NBLK, GB):
                # transpose+halves-add all GB blocks into one PSUM tile,
                # then one copy and one bias-add for the whole group
                ps_log = psum.tile([128, GB * E], FP32, tag="pslt")
                for j in range(j0, j0 + GB):
                    nc.tensor.matmul(
                        ps_log[:, (j - j0) * E : (j - j0 + 1) * E],
                        lhsT=logitsT[:, j * 128 : (j + 1) * 128],
                        rhs=identd[: 2 * E, :],
                        start=True,
                        stop=True,
                    )
                nc.vector.tensor_copy(
                    logit_all[:, j0 : j0 + GB, :],
                    ps_log[:].rearrange("p (g e) -> p g e", e=E),
                )
                biased = small.tile([128, GB, E], FP32, tag="biased")
                nc.vector.tensor_tensor(
                    biased[:], logit_all[:, j0 : j0 + GB, :], bias_sb[:],
                    mybir.AluOpType.add,
                )
                for j in range(j0, j0 + GB):
                    top8 = small.tile([128, 8], FP32, tag="top8")
                    nc.vector.max(top8[:], biased[:, j - j0, :])
                    nc.vector.tensor_scalar(
                        m16_all[:, :, j],
                        biased[:, j - j0, :],
                        top8[:, TOPK - 1 : TOPK],
                        None,
                        op0=mybir.AluOpType.is_ge,
                    )
                    # running cross-block count (prefix over blocks < j)
                    if j == 0:
                        nc.vector.memset(msum_all[:, :, 0], 0.0)
                    else:
                        nc.vector.tensor_tensor(
                            msum_all[:, :, j], msum_all[:, :, j - 1],
                            m16_all[:, :, j - 1], mybir.AluOpType.add,
                        )
                # slot positions for this group
                pos_ps = psum.tile([128, GB * E], FP32, tag="pslt")
                nc.tensor.matmul(
                    pos_ps[:],
                    lhsT=ltri[:],
                    rhs=m16_all[:, :, j0 : j0 + GB],
                    start=True,
                    stop=False,
                )
                nc.tensor.matmul(
                    pos_ps[:],
                    lhsT=lones[:],
                    rhs=msum_all[:, :, j0 : j0 + GB],
                    start=False,
                    stop=True,
                )
                # slot = pos (selected) or >= CAP (masked out)
                slotall = small.tile([128, EPC, GB], FP32, tag="slotall")
                nc.vector.tensor_scalar(
                    slotall[:],
                    m16_all[:, 0:EPC, j0 : j0 + GB],
                    -1.0e6,
                    1.0e6,
                    op0=mybir.AluOpType.mult,
                    op1=mybir.AluOpType.add,
                )
                nc.vector.tensor_tensor(
                    slotall[:],
                    slotall[:],
                    pos_ps[:].rearrange("p (e g) -> p e g", g=GB)[:, 0:EPC, :],
                    mybir.AluOpType.add,
                )
                for e in range(EPC):
                    nc.vector.tensor_scalar_min(
                        slotall[:, e, :], slotall[:, e, :], float(CAP[e])
                    )
                nc.vector.tensor_copy(slotf_all[:, :, j0 : j0 + GB], slotall[:])

                # softmax routing weights for this group
                nc.scalar.activation(
                    expt_all[:, j0 : j0 + GB, :],
                    logit_all[:, j0 : j0 + GB, :],
                    mybir.ActivationFunctionType.Exp,
                )
                for j in range(j0, j0 + GB):
                    nc.vector.tensor_tensor(
                        expt_all[:, j, :],
                        expt_all[:, j, :],
                        m16_all[:, :, j],
                        mybir.AluOpType.mult,
                    )
                nc.vector.reduce_sum(
                    ssum_all[:, j0 : j0 + GB],
                    expt_all[:, j0 : j0 + GB, :],
                    axis=mybir.AxisListType.X,
                )
                nc.vector.reciprocal(
                    rcp_all[:, j0 : j0 + GB], ssum_all[:, j0 : j0 + GB]
                )
                for e in range(EPC):
                    nc.vector.tensor_tensor(
                        tokg[e][:, j0 : j0 + GB, 1],
                        expt_all[:, j0 : j0 + GB, e],
                        rcp_all[:, j0 : j0 + GB],
                        mybir.AluOpType.mult,
                    )

                # one-hot + table matmuls for this group
                for j in range(j0, j0 + GB):
                    for e in range(EPC):
                        CE = CAP[e]
                        oh = ohpool.tile([128, CM], FP16, tag="oh")
                        nc.vector.tensor_scalar(
                            oh[:, :CE],
                            colf[:, :CE],
                            slotf_all[:, e, j : j + 1],
                            None,
                            op0=mybir.AluOpType.is_equal,
                        )
                        nc.tensor.matmul(
                            ps_ta[erow[e] : erow[e] + 2, : min(512, CE)],
                            lhsT=tokg[e][:, j, :],
                            rhs=oh[:, 0 : min(512, CE)],
                            start=(j == 0),
                            stop=(j == NBLK - 1),
                        )
                        if CE > 512:
                            nc.tensor.matmul(
                                ps_tb[e][:],
                                lhsT=tokg[e][:, j, :],
                                rhs=oh[:, 512:CE],
                                start=(j == 0),
                                stop=(j == NBLK - 1),
                            )

            # ---------- tables -> indices -> gathers, per expert ----------
            idxg_t, idxc_t, xg_t, xte_t = [], [], [], []

            def emit_xte(e):
                CE = CAP[e]
                xte = bigpool.tile([128, KO, CE], FP16, tag="xte")
                for s, r0, sz in _slots(CE):
                    for ko in range(KO):
                        ps_x = psum.tile([128, 128], FP16, tag="pslt")
                        nc.tensor.transpose(
                            ps_x[:, :sz],
                            xg_t[e][:sz, s, ko * 128 : (ko + 1) * 128],
                            identf[:sz, :sz],
                        )
                        nc.vector.tensor_copy(
                            xte[:, ko, r0 : r0 + sz], ps_x[:, :sz]
                        )
                xte_t.append(xte)

            for e in range(EPC):
                CE = CAP[e]
                tbl_sb = bpool.tile([2, CE], FP32, tag=f"tbl{e}")
                nc.vector.tensor_copy(
                    tbl_sb[:, 0 : min(512, CE)],
                    ps_ta[erow[e] : erow[e] + 2, 0 : min(512, CE)],
                )
                if CE > 512:
                    nc.vector.tensor_copy(tbl_sb[:, 512:CE], ps_tb[e][:])
                nc.sync.dma_start(tbl[e][:, :], tbl_sb[:])

                nse = len(_slots(CE))
                idxg = bpool.tile([128, nse, 2], FP32, tag=f"idxg{e}")
                idxc = bpool.tile([128, nse], I32, tag=f"idxc{e}")
                xg = xgpool.tile([128, 5, H], FP16, tag="xg")
                for s, r0, sz in _slots(CE):
                    ps_t = psum.tile([128, 2], FP32, tag="pslt")
                    nc.tensor.transpose(
                        ps_t[:sz, :], tbl_sb[:, r0 : r0 + sz], ident32[:2, :2]
                    )
                    nc.vector.tensor_copy(idxg[:sz, s, :], ps_t[:sz, :])
                    nc.vector.tensor_copy(
                        idxc[:sz, s : s + 1], idxg[:sz, s, 0:1]
                    )
                    nc.gpsimd.indirect_dma_start(
                        out=xg[:sz, s, :],
                        out_offset=None,
                        in_=x16[:, :],
                        in_offset=IndirectOffsetOnAxis(
                            ap=idxc[:sz, s : s + 1], axis=0
                        ),
                    )
                idxg_t.append(idxg)
                idxc_t.append(idxc)
                xg_t.append(xg)
                emit_xte(e)

            # ---------- shared expert (fills the dispatch gap) ----------
            for mi in range(II // 128):
                ps_a = psum4.tile([128, 512], FP32, tag="mm")
                for ko in range(KO):
                    nc.tensor.matmul(
                        ps_a[:, :TSH],
                        lhsT=sw1s[:, ko, mi * 128 : (mi + 1) * 128],
                        rhs=xts[:, ko, :],
                        start=(ko == 0),
                        stop=(ko == KO - 1),
                    )
                silu_into(ush[:, mi, :], ps_a[:, :TSH])
                ps_b = psum4.tile([128, 512], FP32, tag="mm")
                for ko in range(KO):
                    nc.tensor.matmul(
                        ps_b[:, :TSH],
                        lhsT=sw3s[:, ko, mi * 128 : (mi + 1) * 128],
                        rhs=xts[:, ko, :],
                        start=(ko == 0),
                        stop=(ko == KO - 1),
                    )
                nc.vector.tensor_tensor(
                    ush[:, mi, :], ush[:, mi, :], ps_b[:, :TSH],
                    mybir.AluOpType.mult,
                )

            for s2 in range(TSH // 128):
                ysh_sb = ypool.tile([128, H], FP16, tag="y")
                for c2 in range(H // 512):
                    ps_y = psum4.tile([128, 512], FP32, tag="mm")
                    for ko in range(KO):
                        nc.tensor.matmul(
                            ps_y[:],
                            lhsT=ush[:, ko, s2 * 128 : (s2 + 1) * 128],
                            rhs=sw2s[:, ko, c2 * 512 : (c2 + 1) * 512],
                            start=(ko == 0),
                            stop=(ko == KO - 1),
                        )
                    nc.vector.tensor_copy(
                        ysh_sb[:, c2 * 512 : (c2 + 1) * 512], ps_y[:]
                    )
                nc.sync.dma_start(ysh[s2 * 128 : (s2 + 1) * 128, :], ysh_sb[:])

            # PE filler during the dispatch window
            wu2_ps = psum4.tile([128, 512], FP32, tag="mm")
            for w in range(6):
                nc.tensor.matmul(
                    wu2_ps[:], lhsT=lones[:], rhs=warm[:],
                    start=(w == 0), stop=(w == 5),
                )
            wu2_sb = small.tile([128, 16], FP32, tag="warm")
            nc.vector.tensor_copy(wu2_sb[:], wu2_ps[:, :16])
            nc.sync.dma_start(trash[0:1, 16:32], wu2_sb[:1, :])

            # ---------- phase B: routed experts ----------
            for e in range(EPC):
                CE = CAP[e]
                xte = xte_t[e]
                if e > 0:
                    with tc.tile_wait_until(0.054):
                        w1s = wpool.tile([128, KO, II], FP16, tag="w1")
                        nc.scalar.dma_start(w1s[:], w1t[e])
                        w3s = wpool.tile([128, KO, II], FP16, tag="w3")
                        nc.scalar.dma_start(w3s[:], w3t[e])
                with tc.tile_wait_until(0.038 if e == 0 else 0.070):
                    w2s = w2pool.tile([128, KO, H], FP16, tag="w2")
                    nc.scalar.dma_start(w2s[:], w2t[e])

                u16 = upool.tile([128, KO, CE], FP16, tag="u16")
                for n0, nw in _seg512(CE):
                    for mi in range(II // 128):
                        ps_a = psum4.tile([128, 512], FP32, tag="mm")
                        for ko in range(KO):
                            nc.tensor.matmul(
                                ps_a[:, :nw],
                                lhsT=w1s[:, ko, mi * 128 : (mi + 1) * 128],
                                rhs=xte[:, ko, n0 : n0 + nw],
                                start=(ko == 0),
                                stop=(ko == KO - 1),
                            )
                        silu_into(u16[:, mi, n0 : n0 + nw], ps_a[:, :nw])
                        ps_b = psum4.tile([128, 512], FP32, tag="mm")
                        for ko in range(KO):
                            nc.tensor.matmul(
                                ps_b[:, :nw],
                                lhsT=w3s[:, ko, mi * 128 : (mi + 1) * 128],
                                rhs=xte[:, ko, n0 : n0 + nw],
                                start=(ko == 0),
                                stop=(ko == KO - 1),
                            )
                        nc.vector.tensor_tensor(
                            u16[:, mi, n0 : n0 + nw],
                            u16[:, mi, n0 : n0 + nw],
                            ps_b[:, :nw],
                            mybir.AluOpType.mult,
                        )

                for s, r0, sz in _slots(CE):
                    y_s = ypool.tile([128, H], FP16, tag="y")
                    for c2 in range(H // 512):
                        ps_y = psum4.tile([128, 512], FP32, tag="mm")
                        for ko in range(KO):
                            nc.tensor.matmul(
                                ps_y[:sz, :],
                                lhsT=u16[:, ko, r0 : r0 + sz],
                                rhs=w2s[:, ko, c2 * 512 : (c2 + 1) * 512],
                                start=(ko == 0),
                                stop=(ko == KO - 1),
                            )
                        # y = psum * g (routing weight), on the Vector engine
                        nc.vector.tensor_scalar_mul(
                            y_s[:sz, c2 * 512 : (c2 + 1) * 512],
                            ps_y[:sz, :],
                            idxg_t[e][:sz, s, 1:2],
                        )
                    nc.sync.dma_start(ye[e][r0 : r0 + sz, :], y_s[:sz, :])

    nc.compile()
    return nc


def _get_nc(caps=(576, 512)):
    key = (bool(USE_SILU), caps)
    if key not in _compiled:
        _compiled[key] = _build(USE_SILU, caps)
    return _compiled[key]


def _plan(x, gate_w, expert_bias):
    """Host-side placement: balanced expert pairing + slot capacities.

    Runs the same top-4 the device computes (margins are huge relative
    to both fp32 and the device's split-fp16 error) purely to decide
    which expert goes in which slot and how much capacity to compile.
    """
    logits = x @ gate_w.T + expert_bias[None, :]
    sel = np.argpartition(-logits, TOPK - 1, axis=1)[:, :TOPK]
    counts = np.bincount(sel.ravel(), minlength=E)
    order = np.argsort(-counts, kind="stable")
    big, small = order[:NCORES], order[NCORES:][::-1]

    def r64(n):
        return max(128, int(-(-n // 64) * 64))

    C0 = r64(int(counts[big].max()))
    C1 = r64(int(counts[small].max()))
    return big, small, (C0, C1)


def make_in_maps(hidden_states, gate_w, expert_bias, w1, w2, w3, sw1, sw2, sw3):
    x = np.asarray(hidden_states, np.float32).reshape(T, H)
    gate_w = np.asarray(gate_w, np.float32)
    expert_bias = np.asarray(expert_bias, np.float32)
    w1 = np.asarray(w1, np.float32)
    w2 = np.asarray(w2, np.float32)
    w3 = np.asarray(w3, np.float32)

    big, small, caps = _plan(x, gate_w, expert_bias)

    def ktile(m):
        # [K, N] -> [ki, ko, N] with contiguous per-partition lines
        return np.ascontiguousarray(
            m.reshape(KO, 128, m.shape[1]).transpose(1, 0, 2)
        )

    # [NCH, 128, KO, CW] fp16 transposed activation chunks
    xch = np.ascontiguousarray(
        x.reshape(NCH, CW, KO, 128).transpose(0, 3, 2, 1)
    ).astype(np.float16)
    x16 = x.astype(np.float16)
    in_maps = []
    for c in range(NCORES):
        own = [int(big[c]), int(small[c])]
        perm = own + [e for e in range(E) if e not in own]
        chorder = [c] + [k for k in range(NCH) if k != c]
        tokmap = (
            256 * np.asarray(chorder, np.int32)[:, None].repeat(2, 1)
            + np.asarray([0, 128], np.int32)[None, :]
        ).reshape(1, NBLK) + np.arange(128, dtype=np.int32)[:, None]
        gp = np.ascontiguousarray(gate_w[perm].T)          # [H, E] fp32
        g16 = gp.astype(np.float16)
        dg16 = (gp - g16.astype(np.float32)).astype(np.float16)
        gcat = np.concatenate([g16, dg16], axis=1)         # [H, 2E] fp16
        in_maps.append(
            {
                "xT16": np.ascontiguousarray(xch[chorder]),
                "x16": x16,
                "tokmap": np.ascontiguousarray(tokmap),
                "gwt": ktile(gcat),
                "identd_in": np.vstack([np.eye(E)] * 8).astype(np.float32),
                "bias_bc": np.tile(expert_bias[perm], (128, 4, 1)),
                "w1t": np.stack(
                    [ktile(w1[e].T.astype(np.float16)) for e in own]
                ),
                "w3t": np.stack(
                    [ktile(w3[e].T.astype(np.float16)) for e in own]
                ),
                "w2t": np.stack(
                    [ktile(w2[e].T.astype(np.float16)) for e in own]
                ),
                "sw1t": ktile(np.asarray(sw1, np.float32).T.astype(np.float16)),
                "sw3t": ktile(np.asarray(sw3, np.float32).T.astype(np.float16)),
                "sw2t": ktile(np.asarray(sw2, np.float32).T.astype(np.float16)),
            }
        )
    return in_maps, caps


def combine(results):
    out = np.zeros((T, H), np.float32)
    for c in range(NCORES):
        r = results[c]
        out[c * TSH : (c + 1) * TSH] += r["ysh"].astype(np.float32)
        for e in range(EPC):
            tb = r[f"tbl{e}"]
            tok = tb[0, :].astype(np.int64)
            gv = tb[1, :]
            m = gv != 0.0
            out[tok[m]] += r[f"ye{e}"][m].astype(np.float32)
    return out.reshape(1, T, H)


def kernel(hidden_states, gate_w, expert_bias, w1, w2, w3, sw1, sw2, sw3, **kw):
    in_maps, caps = make_in_maps(
        hidden_states, gate_w, expert_bias, w1, w2, w3, sw1, sw2, sw3
    )
    nc = _get_nc(caps)
    res = run_bass_kernel_spmd(nc, in_maps, list(range(NCORES)))
    return combine(res.results)


# revision 57
# speedup vs baseline: 1.0674x; 1.0674x over previous
"""Trainium2 Bass kernel for a 16-expert top-4 MoE layer with shared expert.

Strategy (8 NeuronCores, expert-parallel):
  - Each core owns 2 experts. The host pairs a high-count expert with a
    low-count one (balanced pairing) and compiles with slot capacities
    C0/C1 rounded up from the actual max counts (runtime-adaptive
    compile constants), instead of a fixed worst-case capacity.
  - The router is replicated on every core and must reproduce the fp32
    top-4 selection exactly: the gate matrix is sent as a split-fp16
    pair [g16 | dg16] (stationary, 128 x 32) against an fp16 activation
    stream x16.  The residual-x term is dropped (host-verified: the
    selection margin is >> the dropped term for this scheme), so the
    router stream is half the bytes of a full split.
  - Activations stream in 8 chunks of 256 tokens, HOST-PERMUTED so that
    chunk 0 is always the core's own shared-expert token slice; the
    chunk doubles as the shared-expert input (no separate xTs tensor).
    A tokmap input gives the absolute token id per (partition, block).
  - Dispatch tables are built on-chip with the PE: per (block, expert) a
    one-hot matrix from a DVE is_eq against a column-iota, accumulated
    as [tok|g]^T @ onehot in PSUM.  Slot positions come from a
    strict-upper-triangular prefix matmul plus a cross-block running
    count.  Per-block DVE work is batched across blocks where possible.
  - Each expert gathers its C_e token rows (fp16) with a single
    indirect DMA ([128, NS]-offset), transposes them on the PE to
    [H, C_e], computes SwiGLU in fp16, scales rows by the table's
    routing weight on the Vector engine, and writes compact [C_e, H]
    fp16 rows.  The host applies out[tok] += y per expert using the
    [2, C_e] table, so there is no on-device output scatter.
  - The shared expert is token-sliced: core c computes tokens
    [256c, 256(c+1)); its matmuls fill the dispatch window.
"""

import numpy as np

import concourse.bass as bass
import concourse.mybir as mybir
import concourse.tile as tile
from concourse import bacc
from concourse.bass import IndirectOffsetOnAxis
from concourse.bass_utils import run_bass_kernel_spmd
from concourse.masks import make_identity, make_upper_triangular

FP32 = mybir.dt.float32
FP16 = mybir.dt.float16
I32 = mybir.dt.int32

T = 2048
H = 1024
II = 1024  # intermediate size
E = 16
TOPK = 4
NCORES = 8
EPC = 2            # experts per core
TSH = T // NCORES  # shared-expert tokens per core
NCH = 8            # router activation chunks
CW = T // NCH      # tokens per chunk (= TSH)
NBLK = T // 128    # token blocks
KO = H // 128      # contraction subtiles

# The hardware ACT engine has a Silu LUT; CoreSim does not implement it.
USE_SILU = True

_compiled = {}


def _seg512(c):
    """Column segments of width <= 512 covering [0, c)."""
    return [(i, min(512, c - i)) for i in range(0, c, 512)]


def _slots(c):
    """(slot index, row offset, rows) tiles of 128 covering [0, c)."""
    return [(s, s * 128, min(128, c - s * 128)) for s in range((c + 127) // 128)]


def _build(use_silu, caps):
    C0, C1 = caps
    CAP = [C0, C1]
    CM = max(C0, C1)
    nc = bacc.Bacc(None, target_bir_lowering=False, debug=False)

    # ---- I/O ----
    xT16 = nc.dram_tensor("xT16", [NCH, 128, KO, CW], FP16, kind="ExternalInput")
    x16 = nc.dram_tensor("x16", [T, H], FP16, kind="ExternalInput")
    tokmap = nc.dram_tensor("tokmap", [128, NBLK], I32, kind="ExternalInput")
    gwt = nc.dram_tensor("gwt", [128, KO, 2 * E], FP16, kind="ExternalInput")
    identd_in = nc.dram_tensor("identd_in", [128, E], FP32, kind="ExternalInput")
    bias_bc = nc.dram_tensor("bias_bc", [128, 4, E], FP32, kind="ExternalInput")
    w1t = nc.dram_tensor("w1t", [EPC, 128, KO, II], FP16, kind="ExternalInput")
    w3t = nc.dram_tensor("w3t", [EPC, 128, KO, II], FP16, kind="ExternalInput")
    w2t = nc.dram_tensor("w2t", [EPC, 128, KO, H], FP16, kind="ExternalInput")
    sw1t = nc.dram_tensor("sw1t", [128, KO, II], FP16, kind="ExternalInput")
    sw3t = nc.dram_tensor("sw3t", [128, KO, II], FP16, kind="ExternalInput")
    sw2t = nc.dram_tensor("sw2t", [128, KO, H], FP16, kind="ExternalInput")

    ye = [
        nc.dram_tensor(f"ye{e}", [CAP[e], H], FP16, kind="ExternalOutput")
        for e in range(EPC)
    ]
    tbl = [
        nc.dram_tensor(f"tbl{e}", [2, CAP[e]], FP32, kind="ExternalOutput")
        for e in range(EPC)
    ]
    ysh = nc.dram_tensor("ysh", [TSH, H], FP16, kind="ExternalOutput")
    trash = nc.dram_tensor("trash", [1, 512], FP32, kind="ExternalOutput")

    def silu_into(dst, src):
        """dst(f16) = silu(src); src is a PSUM fp32 tile."""
        if use_silu:
            nc.scalar.activation(dst, src, mybir.ActivationFunctionType.Silu)
        else:
            nc.scalar.activation(dst, src, mybir.ActivationFunctionType.Sigmoid)
            nc.vector.tensor_tensor(dst, dst, src, mybir.AluOpType.mult)

    with tile.TileContext(nc) as tc:
        with (
            tc.tile_pool(name="const", bufs=1) as const,
            tc.tile_pool(name="apool", bufs=5) as apool,
            tc.tile_pool(name="small", bufs=3) as small,
            tc.tile_pool(name="state", bufs=1) as state,
            tc.tile_pool(name="wpool", bufs=2) as wpool,
            tc.tile_pool(name="w2pool", bufs=2) as w2pool,
            tc.tile_pool(name="bpool", bufs=2) as bpool,
            tc.tile_pool(name="bigpool", bufs=2) as bigpool,
            tc.tile_pool(name="upool", bufs=2) as upool,
            tc.tile_pool(name="xgpool", bufs=1) as xgpool,
            tc.tile_pool(name="ypool", bufs=1) as ypool,
            tc.tile_pool(name="ohpool", bufs=3) as ohpool,
            tc.tile_pool(name="psum", bufs=2, space="PSUM") as psum,
            tc.tile_pool(name="ptbl", bufs=1, space="PSUM") as ptbl,
            tc.tile_pool(name="psum4", bufs=4, space="PSUM") as psum4,
        ):
            # ---------- critical-path DMAs first ----------
            # chunk 0 doubles as the shared-expert activation slice; it is
            # pinned for the whole kernel.  The sync queue carries ONLY the
            # router activation chunks so they stream back-to-back.
            xts = const.tile([128, KO, CW], FP16)
            nc.sync.dma_start(xts[:], xT16[0])
            gwt_sb = const.tile([128, KO, 2 * E], FP16)
            nc.sync.dma_start(gwt_sb[:], gwt[:, :, :])
            bias_sb = const.tile([128, 4, E], FP32)
            nc.sync.dma_start(bias_sb[:], bias_bc[:, :, :])
            toki = const.tile([128, NBLK], I32)
            nc.gpsimd.dma_start(toki[:], tokmap[:, :])

            # ---------- constants ----------
            lones = const.tile([128, 128], FP16)
            nc.gpsimd.memset(lones[:], 1.0)
            warm = const.tile([128, 512], FP16)
            nc.vector.memset(warm[:], 1.0)
            ltri = const.tile([128, 128], FP16)
            make_upper_triangular(nc, ltri[:], val=1.0, diag=False)  # k<m strictly
            ident32 = const.tile([128, 128], FP32)
            make_identity(nc, ident32[:])
            identf = const.tile([128, 128], FP16)
            make_identity(nc, identf[:])
            # stacked identity (I every 16 rows): the block transpose matmul
            # sums the g16*x and dg16*x logit halves for free, and works from
            # any 32-row band of the packed router PSUM tile
            identd = const.tile([128, E], FP32)
            nc.sync.dma_start(identd[:], identd_in[:, :])

            tokf = const.tile([128, NBLK], FP16)
            nc.vector.tensor_copy(tokf[:], toki[:])
            coli = state.tile([128, CM], I32)
            nc.gpsimd.iota(coli[:], pattern=[[1, CM]], base=0, channel_multiplier=0)
            colf = const.tile([128, CM], FP16)
            nc.vector.tensor_copy(colf[:], coli[:])

            # per-expert [token | routing weight] columns
            tokg = []
            for e in range(EPC):
                tg = state.tile([128, NBLK, 2], FP16, tag=f"tokg{e}")
                nc.vector.tensor_copy(tg[:, :, 0], tokf[:])
                tokg.append(tg)

            ush = bpool.tile([128, KO, TSH], FP16, tag="ush")

            # PE warmup: ramps the clock gate while the chunk-0 DMA lands
            wu_ps = psum4.tile([128, 512], FP32, tag="mm")
            for w in range(12):
                nc.tensor.matmul(
                    wu_ps[:], lhsT=lones[:], rhs=warm[:],
                    start=(w == 0), stop=(w == 11),
                )
            wu_sb = small.tile([128, 16], FP32, tag="warm")
            nc.vector.tensor_copy(wu_sb[:], wu_ps[:, :16])
            nc.sync.dma_start(trash[0:1, :16], wu_sb[:1, :])

            logitsT = state.tile([2 * E, T], FP32)
            logit_all = state.tile([128, NBLK, E], FP32)
            m16_all = state.tile([128, E, NBLK], FP16)
            msum_all = state.tile([128, E, NBLK], FP16)
            expt_all = state.tile([128, NBLK, E], FP32)
            ssum_all = state.tile([128, NBLK], FP32)
            rcp_all = state.tile([128, NBLK], FP32)
            slotf_all = state.tile([128, EPC, NBLK], FP32)

            # ---------- phase A1: router matmuls ----------
            # 4 chunks accumulate into one PSUM tile at 32-row bands; one
            # full-width DVE copy drains each tile
            ps_lt = None
            for ch in range(NCH):
                if ch == 0:
                    xd_c = xts
                else:
                    xd_c = apool.tile([128, KO, CW], FP16, tag="xd")
                    q = [nc.sync, nc.scalar, nc.gpsimd][ch % 3]
                    q.dma_start(xd_c[:], xT16[ch])
                ps_lt = psum.tile([2 * E, CW], FP32, tag="pslt")
                for ko in range(KO):
                    nc.tensor.matmul(
                        ps_lt[:],
                        lhsT=gwt_sb[:, ko, :],
                        rhs=xd_c[:, ko, :],
                        start=(ko == 0),
                        stop=(ko == KO - 1),
                    )
                nc.vector.tensor_copy(
                    logitsT[:, ch * CW : (ch + 1) * CW], ps_lt[:]
                )

            # weight DMAs: min-time paced so their traffic lands behind the
            # router stream. IMPORTANT: emitted in pace order — a paced DMA
            # at the head of the queue blocks everything emitted after it.
            with tc.tile_wait_until(0.013):
                sw1s = wpool.tile([128, KO, II], FP16, tag="w1")
                nc.scalar.dma_start(sw1s[:], sw1t[:, :, :])
            with tc.tile_wait_until(0.016):
                sw3s = wpool.tile([128, KO, II], FP16, tag="w3")
                nc.scalar.dma_start(sw3s[:], sw3t[:, :, :])
            with tc.tile_wait_until(0.019):
                w1s = wpool.tile([128, KO, II], FP16, tag="w1")
                nc.scalar.dma_start(w1s[:], w1t[0])
            with tc.tile_wait_until(0.022):
                w3s = wpool.tile([128, KO, II], FP16, tag="w3")
                nc.scalar.dma_start(w3s[:], w3t[0])
            with tc.tile_wait_until(0.028):
                sw2s = w2pool.tile([128, KO, H], FP16, tag="w2")
                nc.scalar.dma_start(sw2s[:], sw2t[:, :, :])

            # ---------- dispatch: blocks -> positions -> tables, pipelined
            # per group of GB blocks so tables finish right behind the router
            GB = 4
            erow = [0, 32]  # expert row offset in the shared table PSUM bank
            ps_ta = ptbl.tile([erow[-1] + 2, 512], FP32, tag="ta")
            ps_tb = {}
            for e in range(EPC):
                if CAP[e] > 512:
                    ps_tbe = ptbl.tile([2, CAP[e] - 512], FP32, tag=f"tb{e}")
                    ps_tb[e] = ps_tbe

            for j0 in range(0, NBLK, GB):
                # transpose+halves-add all GB blocks into one PSUM tile,
                # then one copy and one bias-add for the whole group
                ps_log = psum.tile([128, GB * E], FP32, tag="pslt")
                for j in range(j0, j0 + GB):
                    nc.tensor.matmul(
                        ps_log[:, (j - j0) * E : (j - j0 + 1) * E],
                        lhsT=logitsT[:, j * 128 : (j + 1) * 128],
                        rhs=identd[: 2 * E, :],
                        start=True,
                        stop=True,
                    )
                nc.vector.tensor_copy(
                    logit_all[:, j0 : j0 + GB, :],
                    ps_log[:].rearrange("p (g e) -> p g e", e=E),
                )
                biased = small.tile([128, GB, E], FP32, tag="biased")
                nc.vector.tensor_tensor(
                    biased[:], logit_all[:, j0 : j0 + GB, :], bias_sb[:],
                    mybir.AluOpType.add,
                )
                for j in range(j0, j0 + GB):
                    top8 = small.tile([128, 8], FP32, tag="top8")
                    nc.vector.max(top8[:], biased[:, j - j0, :])
                    nc.vector.tensor_scalar(
                        m16_all[:, :, j],
                        biased[:, j - j0, :],
                        top8[:, TOPK - 1 : TOPK],
                        None,
                        op0=mybir.AluOpType.is_ge,
                    )
                    # running cross-block count (prefix over blocks < j)
                    if j == 0:
                        nc.vector.memset(msum_all[:, :, 0], 0.0)
                    else:
                        nc.vector.tensor_tensor(
                            msum_all[:, :, j], msum_all[:, :, j - 1],
                            m16_all[:, :, j - 1], mybir.AluOpType.add,
                        )
                # slot positions for this group
                pos_ps = psum.tile([128, GB * E], FP32, tag="pslt")
                nc.tensor.matmul(
                    pos_ps[:],
                    lhsT=ltri[:],
                    rhs=m16_all[:, :, j0 : j0 + GB],
                    start=True,
                    stop=False,
                )
                nc.tensor.matmul(
                    pos_ps[:],
                    lhsT=lones[:],
                    rhs=msum_all[:, :, j0 : j0 + GB],
                    start=False,
                    stop=True,
                )
                # slot = pos (selected) or >= CAP (masked out)
                slotall = small.tile([128, EPC, GB], FP32, tag="slotall")
                nc.vector.tensor_scalar(
                    slotall[:],
                    m16_all[:, 0:EPC, j0 : j0 + GB],
                    -1.0e6,
                    1.0e6,
                    op0=mybir.AluOpType.mult,
                    op1=mybir.AluOpType.add,
                )
                nc.vector.tensor_tensor(
                    slotall[:],
                    slotall[:],
                    pos_ps[:].rearrange("p (e g) -> p e g", g=GB)[:, 0:EPC, :],
                    mybir.AluOpType.add,
                )
                for e in range(EPC):
                    nc.vector.tensor_scalar_min(
                        slotall[:, e, :], slotall[:, e, :], float(CAP[e])
                    )
                nc.vector.tensor_copy(slotf_all[:, :, j0 : j0 + GB], slotall[:])

                # softmax routing weights for this group
                nc.scalar.activation(
                    expt_all[:, j0 : j0 + GB, :],
                    logit_all[:, j0 : j0 + GB, :],
                    mybir.ActivationFunctionType.Exp,
                )
                for j in range(j0, j0 + GB):
                    nc.vector.tensor_tensor(
                        expt_all[:, j, :],
                        expt_all[:, j, :],
                        m16_all[:, :, j],
                        mybir.AluOpType.mult,
                    )
                nc.vector.reduce_sum(
                    ssum_all[:, j0 : j0 + GB],
                    expt_all[:, j0 : j0 + GB, :],
                    axis=mybir.AxisListType.X,
                )
                nc.vector.reciprocal(
                    rcp_all[:, j0 : j0 + GB], ssum_all[:, j0 : j0 + GB]
                )
                for e in range(EPC):
                    nc.vector.tensor_tensor(
                        tokg[e][:, j0 : j0 + GB, 1],
                        expt_all[:, j0 : j0 + GB, e],
                        rcp_all[:, j0 : j0 + GB],
                        mybir.AluOpType.mult,
                    )

                # one-hot + table matmuls for this group
                for j in range(j0, j0 + GB):
                    for e in range(EPC):
                        CE = CAP[e]
                        oh = ohpool.tile([128, CM], FP16, tag="oh")
                        nc.vector.tensor_scalar(
                            oh[:, :CE],
                            colf[:, :CE],
                            slotf_all[:, e, j : j + 1],
                            None,
                            op0=mybir.AluOpType.is_equal,
                        )
                        nc.tensor.matmul(
                            ps_ta[erow[e] : erow[e] + 2, : min(512, CE)],
                            lhsT=tokg[e][:, j, :],
                            rhs=oh[:, 0 : min(512, CE)],
                            start=(j == 0),
                            stop=(j == NBLK - 1),
                        )
                        if CE > 512:
                            nc.tensor.matmul(
                                ps_tb[e][:],
                                lhsT=tokg[e][:, j, :],
                                rhs=oh[:, 512:CE],
                                start=(j == 0),
                                stop=(j == NBLK - 1),
                            )

            # ---------- tables -> indices -> gathers, per expert ----------
            idxg_t, idxc_t, xg_t, xte_t = [], [], [], []

            def emit_xte(e):
                CE = CAP[e]
                xte = bigpool.tile([128, KO, CE], FP16, tag="xte")
                for s, r0, sz in _slots(CE):
                    for ko in range(KO):
                        ps_x = psum.tile([128, 128], FP16, tag="pslt")
                        nc.tensor.transpose(
                            ps_x[:, :sz],
                            xg_t[e][:sz, s, ko * 128 : (ko + 1) * 128],
                            identf[:sz, :sz],
                        )
                        nc.vector.tensor_copy(
                            xte[:, ko, r0 : r0 + sz], ps_x[:, :sz]
                        )
                xte_t.append(xte)

            for e in range(EPC):
                CE = CAP[e]
                tbl_sb = bpool.tile([2, CE], FP32, tag=f"tbl{e}")
                nc.vector.tensor_copy(
                    tbl_sb[:, 0 : min(512, CE)],
                    ps_ta[erow[e] : erow[e] + 2, 0 : min(512, CE)],
                )
                if CE > 512:
                    nc.vector.tensor_copy(tbl_sb[:, 512:CE], ps_tb[e][:])
                nc.sync.dma_start(tbl[e][:, :], tbl_sb[:])

                nse = len(_slots(CE))
                idxg = bpool.tile([128, nse, 2], FP32, tag=f"idxg{e}")
                idxc = bpool.tile([128, nse], I32, tag=f"idxc{e}")
                xg = xgpool.tile([128, 5, H], FP16, tag="xg")
                for s, r0, sz in _slots(CE):
                    ps_t = psum.tile([128, 2], FP32, tag="pslt")
                    nc.tensor.transpose(
                        ps_t[:sz, :], tbl_sb[:, r0 : r0 + sz], ident32[:2, :2]
                    )
                    nc.vector.tensor_copy(idxg[:sz, s, :], ps_t[:sz, :])
                    nc.vector.tensor_copy(
                        idxc[:sz, s : s + 1], idxg[:sz, s, 0:1]
                    )
                    nc.gpsimd.indirect_dma_start(
                        out=xg[:sz, s, :],
                        out_offset=None,
                        in_=x16[:, :],
                        in_offset=IndirectOffsetOnAxis(
                            ap=idxc[:sz, s : s + 1], axis=0
                        ),
                    )
                idxg_t.append(idxg)
                idxc_t.append(idxc)
                xg_t.append(xg)
                emit_xte(e)

            # ---------- shared expert (fills the dispatch gap) ----------
            for mi in range(II // 128):
                ps_a = psum4.tile([128, 512], FP32, tag="mm")
                for ko in range(KO):
                    nc.tensor.matmul(
                        ps_a[:, :TSH],
                        lhsT=sw1s[:, ko, mi * 128 : (mi + 1) * 128],
                        rhs=xts[:, ko, :],
                        start=(ko == 0),
                        stop=(ko == KO - 1),
                    )
                silu_into(ush[:, mi, :], ps_a[:, :TSH])
                ps_b = psum4.tile([128, 512], FP32, tag="mm")
                for ko in range(KO):
                    nc.tensor.matmul(
                        ps_b[:, :TSH],
                        lhsT=sw3s[:, ko, mi * 128 : (mi + 1) * 128],
                        rhs=xts[:, ko, :],
                        start=(ko == 0),
                        stop=(ko == KO - 1),
                    )
                nc.vector.tensor_tensor(
                    ush[:, mi, :], ush[:, mi, :], ps_b[:, :TSH],
                    mybir.AluOpType.mult,
                )

            for s2 in range(TSH // 128):
                ysh_sb = ypool.tile([128, H], FP16, tag="y")
                for c2 in range(H // 512):
                    ps_y = psum4.tile([128, 512], FP32, tag="mm")
                    for ko in range(KO):
                        nc.tensor.matmul(
                            ps_y[:],
                            lhsT=ush[:, ko, s2 * 128 : (s2 + 1) * 128],
                            rhs=sw2s[:, ko, c2 * 512 : (c2 + 1) * 512],
                            start=(ko == 0),
                            stop=(ko == KO - 1),
                        )
                    nc.vector.tensor_copy(
                        ysh_sb[:, c2 * 512 : (c2 + 1) * 512], ps_y[:]
                    )
                nc.sync.dma_start(ysh[s2 * 128 : (s2 + 1) * 128, :], ysh_sb[:])

            # PE filler during the dispatch window
            wu2_ps = psum4.tile([128, 512], FP32, tag="mm")
            for w in range(6):
                nc.tensor.matmul(
                    wu2_ps[:], lhsT=lones[:], rhs=warm[:],
                    start=(w == 0), stop=(w == 5),
                )
            wu2_sb = small.tile([128, 16], FP32, tag="warm")
            nc.vector.tensor_copy(wu2_sb[:], wu2_ps[:, :16])
            nc.sync.dma_start(trash[0:1, 16:32], wu2_sb[:1, :])

            # ---------- phase B: routed experts ----------
            for e in range(EPC):
                CE = CAP[e]
                xte = xte_t[e]
                if e > 0:
                    with tc.tile_wait_until(0.056):
                        w1s = wpool.tile([128, KO, II], FP16, tag="w1")
                        nc.scalar.dma_start(w1s[:], w1t[e])
                        w3s = wpool.tile([128, KO, II], FP16, tag="w3")
                        nc.scalar.dma_start(w3s[:], w3t[e])
                with tc.tile_wait_until(0.040 if e == 0 else 0.072):
                    w2s = w2pool.tile([128, KO, H], FP16, tag="w2")
                    nc.scalar.dma_start(w2s[:], w2t[e])

                u16 = upool.tile([128, KO, CE], FP16, tag="u16")
                for n0, nw in _seg512(CE):
                    for mi in range(II // 128):
                        ps_a = psum4.tile([128, 512], FP32, tag="mm")
                        for ko in range(KO):
                            nc.tensor.matmul(
                                ps_a[:, :nw],
                                lhsT=w1s[:, ko, mi * 128 : (mi + 1) * 128],
                                rhs=xte[:, ko, n0 : n0 + nw],
                                start=(ko == 0),
                                stop=(ko == KO - 1),
                            )
                        silu_into(u16[:, mi, n0 : n0 + nw], ps_a[:, :nw])
                        ps_b = psum4.tile([128, 512], FP32, tag="mm")
                        for ko in range(KO):
                            nc.tensor.matmul(
                                ps_b[:, :nw],
                                lhsT=w3s[:, ko, mi * 128 : (mi + 1) * 128],
                                rhs=xte[:, ko, n0 : n0 + nw],
                                start=(ko == 0),
                                stop=(ko == KO - 1),
                            )
                        nc.vector.tensor_tensor(
                            u16[:, mi, n0 : n0 + nw],
                            u16[:, mi, n0 : n0 + nw],
                            ps_b[:, :nw],
                            mybir.AluOpType.mult,
                        )

                for s, r0, sz in _slots(CE):
                    y_s = ypool.tile([128, H], FP16, tag="y")
                    for c2 in range(H // 512):
                        ps_y = psum4.tile([128, 512], FP32, tag="mm")
                        for ko in range(KO):
                            nc.tensor.matmul(
                                ps_y[:sz, :],
                                lhsT=u16[:, ko, r0 : r0 + sz],
                                rhs=w2s[:, ko, c2 * 512 : (c2 + 1) * 512],
                                start=(ko == 0),
                                stop=(ko == KO - 1),
                            )
                        # y = psum * g (routing weight), on the Vector engine
                        nc.vector.tensor_scalar_mul(
                            y_s[:sz, c2 * 512 : (c2 + 1) * 512],
                            ps_y[:sz, :],
                            idxg_t[e][:sz, s, 1:2],
                        )
                    nc.sync.dma_start(ye[e][r0 : r0 + sz, :], y_s[:sz, :])

    nc.compile()
    return nc


def _get_nc(caps=(576, 512)):
    key = (bool(USE_SILU), caps)
    if key not in _compiled:
        _compiled[key] = _build(USE_SILU, caps)
    return _compiled[key]


def _plan(x, gate_w, expert_bias):
    """Host-side placement: balanced expert pairing + slot capacities.

    Runs the same top-4 the device computes (margins are huge relative
    to both fp32 and the device's split-fp16 error) purely to decide
    which expert goes in which slot and how much capacity to compile.
    """
    logits = x @ gate_w.T + expert_bias[None, :]
    sel = np.argpartition(-logits, TOPK - 1, axis=1)[:, :TOPK]
    counts = np.bincount(sel.ravel(), minlength=E)
    order = np.argsort(-counts, kind="stable")
    big, small = order[:NCORES], order[NCORES:][::-1]

    def r64(n):
        return max(128, int(-(-n // 64) * 64))

    C0 = r64(int(counts[big].max()))
    C1 = r64(int(counts[small].max()))
    return big, small, (C0, C1)


def make_in_maps(hidden_states, gate_w, expert_bias, w1, w2, w3, sw1, sw2, sw3):
    x = np.asarray(hidden_states, np.float32).reshape(T, H)
    gate_w = np.asarray(gate_w, np.float32)
    expert_bias = np.asarray(expert_bias, np.float32)
    w1 = np.asarray(w1, np.float32)
    w2 = np.asarray(w2, np.float32)
    w3 = np.asarray(w3, np.float32)

    big, small, caps = _plan(x, gate_w, expert_bias)

    def ktile(m):
        # [K, N] -> [ki, ko, N] with contiguous per-partition lines
        return np.ascontiguousarray(
            m.reshape(KO, 128, m.shape[1]).transpose(1, 0, 2)
        )

    # [NCH, 128, KO, CW] fp16 transposed activation chunks
    xch = np.ascontiguousarray(
        x.reshape(NCH, CW, KO, 128).transpose(0, 3, 2, 1)
    ).astype(np.float16)
    x16 = x.astype(np.float16)
    in_maps = []
    for c in range(NCORES):
        own = [int(big[c]), int(small[c])]
        perm = own + [e for e in range(E) if e not in own]
        chorder = [c] + [k for k in range(NCH) if k != c]
        tokmap = (
            256 * np.asarray(chorder, np.int32)[:, None].repeat(2, 1)
            + np.asarray([0, 128], np.int32)[None, :]
        ).reshape(1, NBLK) + np.arange(128, dtype=np.int32)[:, None]
        gp = np.ascontiguousarray(gate_w[perm].T)          # [H, E] fp32
        g16 = gp.astype(np.float16)
        dg16 = (gp - g16.astype(np.float32)).astype(np.float16)
        gcat = np.concatenate([g16, dg16], axis=1)         # [H, 2E] fp16
        in_maps.append(
            {
                "xT16": np.ascontiguousarray(xch[chorder]),
                "x16": x16,
                "tokmap": np.ascontiguousarray(tokmap),
                "gwt": ktile(gcat),
                "identd_in": np.vstack([np.eye(E)] * 8).astype(np.float32),
                "bias_bc": np.tile(expert_bias[perm], (128, 4, 1)),
                "w1t": np.stack(
                    [ktile(w1[e].T.astype(np.float16)) for e in own]
                ),
                "w3t": np.stack(
                    [ktile(w3[e].T.astype(np.float16)) for e in own]
                ),
                "w2t": np.stack(
                    [ktile(w2[e].T.astype(np.float16)) for e in own]
                ),
                "sw1t": ktile(np.asarray(sw1, np.float32).T.astype(np.float16)),
                "sw3t": ktile(np.asarray(sw3, np.float32).T.astype(np.float16)),
                "sw2t": ktile(np.asarray(sw2, np.float32).T.astype(np.float16)),
            }
        )
    return in_maps, caps


def combine(results):
    out = np.zeros((T, H), np.float32)
    for c in range(NCORES):
        r = results[c]
        out[c * TSH : (c + 1) * TSH] += r["ysh"].astype(np.float32)
        for e in range(EPC):
            tb = r[f"tbl{e}"]
            tok = tb[0, :].astype(np.int64)
            gv = tb[1, :]
            m = gv != 0.0
            out[tok[m]] += r[f"ye{e}"][m].astype(np.float32)
    return out.reshape(1, T, H)


def kernel(hidden_states, gate_w, expert_bias, w1, w2, w3, sw1, sw2, sw3, **kw):
    in_maps, caps = make_in_maps(
        hidden_states, gate_w, expert_bias, w1, w2, w3, sw1, sw2, sw3
    )
    nc = _get_nc(caps)
    res = run_bass_kernel_spmd(nc, in_maps, list(range(NCORES)))
    return combine(res.results)
